# revision 36
# baseline (speedup 1.0000x reference)
"""Trainium2 Bass kernel for nn_Attention_884763263569.

Per-sample compute: k/v projections per view t, q over the concat, 3-way
softmax attention, small FC head.  Pure data-parallel over 8 NeuronCores.

Design (measured ~90-100us vs 115.8us baseline; stream runs at ~420+ GB/s):
 - feature-major stage 1: weights are the STATIONARY operand, x streams
   as the moving operand (N=512), so the PE ingests each x element once
   at stream rate instead of paying a stationary reload per 128 samples.
 - Wfc is folded into Wv on host (f = (Wfc@Wv) x): stage-1 banks carry
   [k 0:32 | qp 32:64 | f 64:74]; stationary padded to 128 cols for FWL.
 - per 512-sample slab: 12 matmuls accumulate three per-view PSUM banks
   (4-deep single-bank rotation); banks evacuate to SBUF bf16 (ACT+DVE),
   then 12 bf16 PE transposes land sample-major in a per-pair PSUM ct
   buffer (double-buffered, [128, 8, 256] so every transpose stays in
   one bank and (slab, block) is one uniform-stride dim).
 - softmax epilogue at 2-slab granularity straight out of PSUM (one
   reduce for q, mult+reduce logits, exp on ACT, recip, weighted-sum,
   scale+bias into a persistent out buffer).
 - PE instruction stream is software-pipelined: transposes of slab N are
   emitted behind the projections of slab N+1 so the strict PE FIFO
   never waits on the PSUM->SBUF copies.
 - x DMAs ride the two HWDGE rings only (SWDGE involvement measured
   ~15% slower aggregate). EVERY slab is chunk-split half per ring, so
   slab arrivals are uniform (~3.7us cadence) instead of per-ring
   bursts that drift into phase and leave >3.4us PE holes (HAM). ONE
   output DMA at the end: a late DMA must be last on its
   completion-semaphore lane, or mid-stream consumers stall on it
   (Tile assigns DMA-completion sems round-robin over 8 lanes).
 - dummy-matmul warmup + fill fillers keep the PE HAM clock gate open
   (idle >~3.4us rethrottles the PE to 1.2GHz).
 - host packs x as [slab, 128, chunk, sample] bf16 (free), and inverts
   the kernel's natural sample permutation on the way out (free).
"""

import os
import sys
from contextlib import ExitStack

import numpy as np

sys.path.insert(0, "/opt/trn_rl_repo")

import ml_dtypes

import concourse.bass as bass
import concourse.tile as tile
from concourse import mybir
from concourse.bass_utils import run_bass_kernel_spmd
from concourse.masks import make_identity

# bass_utils imports antenv.axon_hooks unguarded when BASS_TRACE is set; some
# images ship an antenv without that module — stub it so tracing degrades
# gracefully instead of crashing.
try:
    import antenv.axon_hooks  # noqa: F401
except ImportError:
    import types

    import antenv

    _hooks = types.ModuleType("antenv.axon_hooks")
    _hooks._h = None
    _hooks.set_axon_ntff_profile_hook = lambda h: setattr(_hooks, "_h", h)
    _hooks.get_axon_ntff_profile_hook = lambda: _hooks._h
    sys.modules["antenv.axon_hooks"] = _hooks
    antenv.axon_hooks = _hooks


def _register_ctypes_ntff_hook():
    """If no NTFF profile hook is registered, drive profiling via direct
    ctypes calls into libaxon_pjrt.so (slim equivalent of axon.trn's hook;
    same C ABI the boot script uses)."""
    import contextlib
    import ctypes

    from antenv.axon_hooks import (
        get_axon_ntff_profile_hook,
        set_axon_ntff_profile_hook,
    )

    if get_axon_ntff_profile_hook() is not None:
        return
    so_path = os.environ.get("AXON_PJRT_SO", "/opt/axon/libaxon_pjrt.so")
    if not os.path.exists(so_path):
        return
    try:
        lib = ctypes.CDLL(so_path)
    except OSError:
        return
    if not hasattr(lib, "axon_start_nrt_profile"):
        return
    lib.axon_start_nrt_profile.argtypes = [
        ctypes.POINTER(ctypes.c_int64),
        ctypes.c_size_t,
    ]
    lib.axon_start_nrt_profile.restype = ctypes.c_int64
    lib.axon_stop_nrt_profile.argtypes = [ctypes.c_char_p]
    lib.axon_stop_nrt_profile.restype = ctypes.c_int64

    @contextlib.contextmanager
    def _hook(output_dir, device_ids):
        import jax

        jax.devices()
        if device_ids:
            ids = (ctypes.c_int64 * len(device_ids))(*device_ids)
            rc = lib.axon_start_nrt_profile(ids, len(device_ids))
        else:
            rc = lib.axon_start_nrt_profile(None, 0)
        if rc != 0:
            raise RuntimeError(f"axon_start_nrt_profile rc={rc}")
        try:
            yield
        finally:
            n = lib.axon_stop_nrt_profile(str(output_dir).encode())
            print(f"ntff profile: {n} file(s) written to {output_dir}", file=sys.stderr)

    set_axon_ntff_profile_hook(_hook)


try:
    _register_ctypes_ntff_hook()
except Exception:
    pass

BF16 = ml_dtypes.bfloat16

NCORES = 8
T, D, P, C = 3, 512, 32, 10
DF = T * D            # 1536
KC = DF // 128        # 12 d-chunks
SLAB = 512            # samples per slab (one matmul moving width)
NW = 74               # useful stationary cols: 32 k + 32 qp + 10 f
WARMUP_MM = 32        # dummy matmuls to open the HAM clock gate


def _ins_dim(ap_obj, pos, size, stride=0):
    """Return a new AP with a [stride, size] dim inserted at position pos."""
    new_ap = [list(d) for d in ap_obj.ap]
    new_ap.insert(pos, [stride, size])
    return bass.AP(tensor=ap_obj.tensor, offset=ap_obj.offset, ap=new_ap)


def _remake_ap(ap_obj, dims):
    """Replace the free dims of an AP (keep partition dim)."""
    new_ap = [list(ap_obj.ap[0])] + [list(d) for d in dims]
    return bass.AP(tensor=ap_obj.tensor, offset=ap_obj.offset, ap=new_ap)


def build_nc(nb):
    assert nb % (2 * SLAB) == 0
    nslabs = nb // SLAB
    npairs = nslabs // 2

    nc = bass.Bass(target_bir_lowering=False)
    xt = nc.declare_dram_parameter(
        "xt", [nslabs, 128, KC, SLAB], mybir.dt.bfloat16, isOutput=False
    )
    wc = nc.declare_dram_parameter("wc", [128, KC, 128], mybir.dt.bfloat16, isOutput=False)
    bfcr = nc.declare_dram_parameter("bfcr", [128, C], mybir.dt.float32, isOutput=False)
    out = nc.declare_dram_parameter(
        "out", [128, nslabs * 4, C], mybir.dt.float32, isOutput=True
    )

    f32 = mybir.dt.float32
    bf16 = mybir.dt.bfloat16
    mult = mybir.AluOpType.mult
    add = mybir.AluOpType.add

    with ExitStack() as ctx:
        tc = ctx.enter_context(tile.TileContext(nc))
        wpool = ctx.enter_context(tc.tile_pool(name="wpool", bufs=1))
        fpool = ctx.enter_context(tc.tile_pool(name="fpool", bufs=1))
        xpool = ctx.enter_context(tc.tile_pool(name="xpool", bufs=6))
        ypsum = ctx.enter_context(tc.tile_pool(name="ypsum", bufs=4, space="PSUM"))
        cpsum = ctx.enter_context(tc.tile_pool(name="cpsum", bufs=1, space="PSUM"))
        ypool = ctx.enter_context(tc.tile_pool(name="ypool", bufs=3))
        spool = ctx.enter_context(tc.tile_pool(name="spool", bufs=2))

        xt_ap = xt.ap()

        # --- weights split across both HWDGE rings (tiny, ~0.9us each) so
        # the x stream starts immediately after on each ring ---
        wc_sb = wpool.tile([128, KC, 128], bf16)
        nc.sync.dma_start(out=wc_sb[:, 0:6, :], in_=wc.ap()[:, 0:6])
        nc.scalar.dma_start(out=wc_sb[:, 6:12, :], in_=wc.ap()[:, 6:12])
        xs0a = fpool.tile([128, KC // 2, SLAB], bf16)
        xs0b = fpool.tile([128, KC // 2, SLAB], bf16)
        nc.sync.dma_start(out=xs0a[:], in_=xt_ap[0, :, 0:6])
        nc.scalar.dma_start(out=xs0b[:], in_=xt_ap[0, :, 6:12])
        bfc_sb = wpool.tile([128, C], f32)
        nc.gpsimd.dma_start(out=bfc_sb[:], in_=bfcr.ap())
        xs1a = fpool.tile([128, KC // 2, SLAB], bf16)
        xs1b = fpool.tile([128, KC // 2, SLAB], bf16)
        nc.sync.dma_start(out=xs1a[:], in_=xt_ap[1, :, 0:6])
        nc.scalar.dma_start(out=xs1b[:], in_=xt_ap[1, :, 6:12])

        ident = wpool.tile([128, 128], bf16)
        make_identity(nc, ident[:])
        obuf = wpool.tile([128, nslabs * 4, C], f32)

        # single ct buffer reused across pairs (bufs=1 semantics via deps)
        # layout: [128, b8=(sl*4+b), 256] bf16; per-block t*74+col, 256-padded
        # so every transpose stays inside one PSUM bank and (sl,b) is one
        # uniform-stride dim for the epilogue APs.
        ct_bufs = [
            cpsum.tile([128, 8, 256], bf16, name="ct_a"),
            cpsum.tile([128, 8, 256], bf16, name="ct_b"),
        ]

        # --- PE warmup: regular matmuls open the HAM clock gate while the
        # first x slabs stream in (transpose-mode would not count as busy)
        warm_ps = ypsum.tile([128, SLAB], f32, name="y_ps")
        for i in range(WARMUP_MM):
            nc.tensor.matmul(
                warm_ps[:, 0:128],
                ident[:],
                ident[:],
                start=True,
                stop=True,
                skip_group_check=True,
            )

        def chunk_ap(sl, c):
            """moving-operand AP for chunk c of slab sl."""
            if sl == 0:
                return (xs0a if c < 6 else xs0b)[:, c % 6, :]
            if sl == 1:
                return (xs1a if c < 6 else xs1b)[:, c % 6, :]
            t = xtiles[sl]
            if isinstance(t, tuple):
                return (t[0] if c < 6 else t[1])[:, c % 6, :]
            return t[:, c, :]

        xtiles = {}
        ysbs = {}
        out_ap = out.ap()

        def prefetch(pf):
            # every slab rides BOTH HWDGE rings (half each): slab arrivals
            # become uniform (~3.7us cadence) instead of per-ring bursts
            # that can drift into phase and leave >3.4us PE holes (HAM).
            if 2 <= pf < nslabs and pf not in xtiles:
                xs = xpool.tile([128, KC, SLAB], bf16, name="xs")
                nc.sync.dma_start(out=xs[:, 0:6, :], in_=xt_ap[pf, :, 0:6])
                nc.scalar.dma_start(out=xs[:, 6:12, :], in_=xt_ap[pf, :, 6:12])
                xtiles[pf] = xs

        prefetch(2)
        prefetch(3)

        def emit_proj(sl):
            # prefetch four ahead on the matching ring (2,3 done at start)
            prefetch(sl + 4)
            y_t = []
            for t in range(T):
                y_ps = ypsum.tile([128, SLAB], f32, name="y_ps")
                y_t.append(y_ps)
                for i in range(4):
                    c = 4 * t + i
                    nc.tensor.matmul(
                        y_ps[:],
                        wc_sb[:, c, :],
                        chunk_ap(sl, c),
                        start=(i == 0),
                        stop=(i == 3),
                    )
            # evacuate banks to SBUF bf16 (ACT 2, DVE 1)
            ysb = ypool.tile([128, T, SLAB], bf16, name="ysb")
            nc.scalar.copy(out=ysb[:, 0, :], in_=y_t[0][:])
            nc.vector.tensor_copy(out=ysb[:, 1, :], in_=y_t[1][:])
            nc.scalar.copy(out=ysb[:, 2, :], in_=y_t[2][:])
            ysbs[sl] = ysb

        def emit_transposes(sl):
            ct_ps = ct_bufs[(sl // 2) % 2]
            half = sl % 2
            ysb = ysbs.pop(sl)
            for b in range(4):
                for t in range(T):
                    nc.tensor.transpose(
                        ct_ps[:, half * 4 + b, t * NW : t * NW + NW],
                        ysb[0:NW, t, b * 128 : (b + 1) * 128],
                        ident[0:NW, 0:NW],
                    )

        def emit_epilogue(pair, h0=0, nb8=8):
            ct_ps = ct_bufs[pair % 2]

            def ctv(c0, c1):
                return _remake_ap(
                    ct_ps[:, h0, c0:c1], [[256, nb8], [NW, 3], [1, c1 - c0]]
                )

            # q[b8,p] = sum_t qp[b8,t,p] — one reduce with t innermost
            q = spool.tile([128, nb8, P], f32, name="q")
            qp_x = _remake_ap(
                ct_ps[:, h0, 32:64], [[256, nb8], [1, P], [NW, T]]
            )
            nc.vector.tensor_reduce(
                out=q[:], in_=qp_x, axis=mybir.AxisListType.X, op=add
            )

            # m[b8,t,p] = q[b8,p] * k[b8,t,p]
            m = spool.tile([128, nb8, T, P], f32, name="m")
            q_b = _ins_dim(q[:], 2, T, 0)
            nc.vector.tensor_tensor(out=m[:], in0=q_b, in1=ctv(0, 32), op=mult)
            logits = spool.tile([128, nb8, T], f32, name="l")
            nc.vector.tensor_reduce(
                out=logits[:], in_=m[:], axis=mybir.AxisListType.X, op=add
            )

            # e = exp(logits) (logits bounded ~±35, no max-subtraction needed)
            e = spool.tile([128, nb8, T], f32, name="e")
            nc.scalar.activation(
                out=e[:], in_=logits[:], func=mybir.ActivationFunctionType.Exp
            )
            z = spool.tile([128, nb8, 1], f32, name="z")
            nc.vector.tensor_reduce(out=z[:], in_=e[:], axis=mybir.AxisListType.X, op=add)
            r = spool.tile([128, nb8, 1], f32, name="r")
            nc.vector.reciprocal(out=r[:], in_=z[:])

            # s[b8,f,t] = e[b8,t] * fmat[b8,t,f]  (written t-innermost)
            s = spool.tile([128, nb8, C, T], f32, name="s")
            e_b = _ins_dim(e[:], 3, C, 0)
            s_out = _remake_ap(s[:], [[C * T, nb8], [1, T], [T, C]])
            nc.vector.tensor_tensor(out=s_out, in0=e_b, in1=ctv(64, 74), op=mult)
            u = spool.tile([128, nb8, C], f32, name="u")
            nc.vector.tensor_reduce(out=u[:], in_=s[:], axis=mybir.AxisListType.X, op=add)

            # out = u * r + bfc
            un = spool.tile([128, nb8, C], f32, name="un")
            r_b = _ins_dim(r[:, :, 0], 2, C, 0)
            nc.vector.tensor_tensor(out=un[:], in0=u[:], in1=r_b, op=mult)
            bfc_b = _ins_dim(bfc_sb[:], 1, nb8, 0)
            nc.vector.tensor_tensor(
                out=obuf[:, pair * 8 + h0 : pair * 8 + h0 + nb8, :],
                in0=un[:],
                in1=bfc_b,
                op=add,
            )


        # software-pipelined emission: transposes for slab N ride behind
        # the projections of slab N+1 so the PE FIFO never waits on the
        # PSUM->SBUF copies.
        def fill_filler(n):
            for _ in range(n):
                nc.tensor.matmul(
                    warm_ps[:, 0:128],
                    ident[:],
                    ident[:],
                    start=True,
                    stop=True,
                    skip_group_check=True,
                )

        emit_proj(0)
        for sl in range(1, nslabs):
            emit_proj(sl)
            emit_transposes(sl - 1)
            if sl <= 3:
                fill_filler(16)
            if sl >= 2 and sl % 2 == 0:
                emit_epilogue(sl // 2 - 1)

        emit_transposes(nslabs - 1)
        emit_epilogue(npairs - 1)
        nc.sync.dma_start(out=out_ap[:], in_=obuf[:])

    nc.finalize()
    _split_excess_waits(nc)
    return nc


def _split_excess_waits(nc):
    """walrus rejects >1 sync wait on compute instruction structs; hoist the
    extras onto same-engine NoOps inserted just before the offender."""
    exempt = (mybir.InstEventSemaphore,)
    for func in nc.m.functions:
        for blk in func.blocks:
            insts = list(blk.instructions)
            out_list = []
            changed = False
            for inst in insts:
                si = getattr(inst, "sync_info", None)
                ow = list(si.on_wait) if (si is not None and si.on_wait) else []
                if len(ow) > 1 and not isinstance(inst, exempt):
                    for w in ow[:-1]:
                        nop = mybir.InstNoOp(
                            name=nc.get_next_instruction_name(),
                            engine=inst.engine,
                            sync_info=mybir.SyncInfo(on_wait=[w], on_update=[]),
                            bass_nofuse=True,
                        )
                        out_list.append(nop)
                    si.on_wait = [ow[-1]]
                    changed = True
                out_list.append(inst)
            if changed:
                blk.instructions = out_list


_NC_CACHE = {}


def _get_nc(nb):
    if nb not in _NC_CACHE:
        _NC_CACHE[nb] = build_nc(nb)
    return _NC_CACHE[nb]


def _prep_weights(Wk, Wv, Wq, Wfc, bfc):
    Wvf = (Wfc.astype(np.float64) @ Wv.astype(np.float64)).astype(np.float32)  # [10,512]
    WkT = Wk.T.astype(np.float32)    # [512, 32]
    WqT = Wq.T.astype(np.float32)    # [1536, 32]
    WvfT = Wvf.T                     # [512, 10]
    wc = np.zeros((KC, 128, 128), np.float32)
    for c in range(KC):
        t, dsub = divmod(c, 4)
        d512 = slice(dsub * 128, (dsub + 1) * 128)
        rows = slice(c * 128, (c + 1) * 128)
        wc[c, :, 0:32] = WkT[d512]
        wc[c, :, 32:64] = WqT[rows]
        wc[c, :, 64:74] = WvfT[d512]
    wc = np.ascontiguousarray(wc.transpose(1, 0, 2)).astype(BF16)  # [128, KC, 128]
    bfcr = np.ascontiguousarray(
        np.broadcast_to(bfc.reshape(1, C).astype(np.float32), (128, C))
    )
    return wc, bfcr


def _pack_x(xr_core, nb):
    # xt[h, p, c, s] = x_cat[h*SLAB + s, 128c + p]
    return np.ascontiguousarray(
        xr_core.astype(BF16)
        .T.reshape(KC, 128, nb // SLAB, SLAB)
        .transpose(2, 1, 0, 3)
    )


def _unpack_out(arr, nb):
    # arr [128, nslabs*4, C]; sample s = h*SLAB + b*128 + p -> arr[p, h*4+b]
    nslabs = nb // SLAB
    return (
        arr.reshape(128, nslabs, 4, C).transpose(1, 2, 0, 3).reshape(nb, C)
    )


LAST_RESULT = None


def kernel(x, Wk, Wv, Wq, Wfc, bfc):
    global LAST_RESULT
    x = np.asarray(x, dtype=np.float32)
    Wk = np.asarray(Wk, dtype=np.float32)
    Wv = np.asarray(Wv, dtype=np.float32)
    Wq = np.asarray(Wq, dtype=np.float32)
    Wfc = np.asarray(Wfc, dtype=np.float32)
    bfc = np.asarray(bfc, dtype=np.float32)

    B = x.shape[0]
    assert B % NCORES == 0
    nb = B // NCORES
    nc = _get_nc(nb)
    wc, bfcr = _prep_weights(Wk, Wv, Wq, Wfc, bfc)

    xr = x.reshape(NCORES, nb, DF)
    in_maps = []
    for i in range(NCORES):
        in_maps.append({"xt": _pack_x(xr[i], nb), "wc": wc, "bfcr": bfcr})

    LAST_RESULT = run_bass_kernel_spmd(nc, in_maps, core_ids=list(range(NCORES)))
    res = LAST_RESULT.results
    out = np.concatenate(
        [_unpack_out(res[i]["out"], nb) for i in range(NCORES)], axis=0
    )
    return out.astype(np.float32)
